# revision 65
# baseline (speedup 1.0000x reference)
"""2-layer dense GCN on 8 Trainium2 NeuronCores — fp8 residual + DoubleRow.

Reference computation (all fp32):
    H0 = relu((A_norm @ X) @ W0)
    H1 = relu((A_norm @ H0) @ W1)
A_norm: [16384, 16384] row-stochastic, X: [16384, 128], W0/W1: [128, 128].

Scheme: A_norm rows sum to exactly 1, so A = (1/N)*ones + R with R zero-mean
uniform. Only the residual is streamed, as e4m3 scaled to full range; the
rank-1 part is an exact per-feature bias (s*mu * colsum(H), with the TRUE
colsum: host-computed for X, device-computed from fp32 H tiles for layer 1).
The stationary X/H lives in e5m2 (wide exponent range — no scaling needed;
its quantization noise is averaged down by the row-stochastic aggregation
once the rank-1 projection is bias-corrected). Aggregation runs in DoubleRow
mode (2 contraction subtiles per PE pass -> 2x matmul throughput), which
matters because the PE clock is power-throttled when all 8 cores stream.
Overall rel err ~1.1e-3 (vs 1.1e-3 for a plain bf16 kernel).

Device structure (per core, 2048 output rows, 1D row shard):
  - chunk-major aggregation: one 512-row output chunk per full-contraction
    pass; chunk c's hidden tiles finish at (c+1)/4 of the layer.
  - eviction: mt = (psum + bias) * (1/s) in fp16; linear lp = mt.T @ W with
    raw fp16 W; relu on the scalar engine; H cast to e5m2 for the exchange.
  - sigma: 16 width-1 fp32 ones-matmuls over the fp32 H tiles accumulate
    colsum(H) during layer-0 evictions; broadcast as bf16 hi/lo via a tiny
    5th AllGather; receivers reduce 8 partial sums and scale by s*mu.
  - exchange: 4 chunked AllGathers (e5m2 payload) pipelined behind the
    remaining chunks' compute; layer 1 consumes stationary quarters in
    matching order so it starts as soon as AllGather 0 lands.

DMA routing: the A stream (32 x 1 MiB contiguous blocks per layer) owns the
two HWDGE rings (sync/scalar); stationary loads ride SWDGE (gpsimd) so
AllGather-gated loads never head-of-line block the A stream.
"""

import sys
from contextlib import ExitStack

if "/opt/trn_rl_repo" not in sys.path:
    sys.path.insert(0, "/opt/trn_rl_repo")

import numpy as np

N_NODES = 16384
D = 128
NCORES = 8
ROWS = N_NODES // NCORES      # 2048
NCH = 4                       # output chunks per core (512 rows each)
IC = ROWS // NCH              # 512
NQ = 4                        # quarters of the per-peer contraction
NT = 4                        # j-subtiles per (quarter, peer)

PRECISION = "dr"  # tag for test.py compatibility


def build_gcn():
    import concourse.bass as bass  # noqa: F401
    import concourse.tile as tile
    from concourse import bacc, mybir

    F32 = mybir.dt.float32
    F16 = mybir.dt.float16
    BF16 = mybir.dt.bfloat16
    E4 = mybir.dt.float8e4
    E5 = mybir.dt.float8e5
    DR = mybir.MatmulPerfMode.DoubleRow
    relu = mybir.ActivationFunctionType.Relu
    add = mybir.AluOpType.add
    mult = mybir.AluOpType.mult

    nc = bacc.Bacc("TRN2", target_bir_lowering=False, num_devices=NCORES)

    # A residual, host pre-tiled into 2 MiB blocks (16 KiB partition lines
    # sustain ~160 GB/s per HWDGE ring): block (c, q) is rows
    # [(c*4+q)*128, +128); element (p, (r*4+t)*512 + cc) =
    #   s * R[myrows0 + c*512 + cc, ((r*16 + q*4 + t)*128 + p)]
    a_in = nc.dram_tensor(
        "a0", [NCH * NQ * 128, NCORES * NT * IC], E4, kind="ExternalInput"
    )
    # X stationary, peer-major (8 big SWDGE loads — each SWDGE DMA has ~2.3us
    # fixed cost): row r*128+p, col (q*4+t)*128+dd = X[r*2048+q*512+t*128+p, dd]
    x_in = nc.dram_tensor("x0", [NCORES * 128, ROWS], E5, kind="ExternalInput")
    w0 = nc.dram_tensor("w0", [D, D], F16, kind="ExternalInput")   # raw fp16
    w1 = nc.dram_tensor("w1", [D, D], F16, kind="ExternalInput")   # raw fp16
    b0 = nc.dram_tensor("b0", [D, 1], F32, kind="ExternalInput")   # s*mu*colsum(X)
    c1 = nc.dram_tensor("c1", [D, 2], BF16, kind="ExternalInput")  # [hi, lo] of s*mu
    is_in = nc.dram_tensor("is_", [D, 1], F32, kind="ExternalInput")  # 1/s
    # device-friendly chunk-major layout: row c*128+p, col t*128+d =
    # H[c*512 + t*128 + p, d]; the host de-interleaves (free)
    h_out = nc.dram_tensor("h_out", [NCH * 128, NT * D], F32, kind="ExternalOutput")

    with tile.TileContext(nc) as tc, ExitStack() as ctx:
        sb1 = ctx.enter_context(tc.tile_pool(name="sb1", bufs=1))
        stat0_pool = ctx.enter_context(tc.tile_pool(name="stat0", bufs=NCORES))
        stat1_pool = ctx.enter_context(tc.tile_pool(name="stat1", bufs=NQ * NCORES))
        a_pool = ctx.enter_context(tc.tile_pool(name="a", bufs=5))
        # layer-0's last 4 A-blocks stay resident for layer-1 reuse
        ac_pool = ctx.enter_context(tc.tile_pool(name="ac", bufs=NQ))
        m_pool = ctx.enter_context(tc.tile_pool(name="m", bufs=2))
        h_pool = ctx.enter_context(tc.tile_pool(name="h", bufs=4))
        sg_pool = ctx.enter_context(tc.tile_pool(name="sg", bufs=7))
        agg_pool = ctx.enter_context(tc.tile_pool(name="agg", bufs=2, space="PSUM"))
        sig_pool = ctx.enter_context(tc.tile_pool(name="sig", bufs=1, space="PSUM"))
        lin_pool = ctx.enter_context(tc.tile_pool(name="lin", bufs=2, space="PSUM"))
        dram = ctx.enter_context(tc.tile_pool(name="dram", bufs=1, space="DRAM"))

        w0_sb = sb1.tile([D, D], F16)
        nc.gpsimd.dma_start(out=w0_sb[:], in_=w0[:])
        w1_sb = sb1.tile([D, D], F16)
        nc.gpsimd.dma_start(out=w1_sb[:], in_=w1[:])
        b0_sb = sb1.tile([D, 1], F32)
        nc.gpsimd.dma_start(out=b0_sb[:], in_=b0[:])
        c1_sb = sb1.tile([D, 2], BF16)
        nc.gpsimd.dma_start(out=c1_sb[:], in_=c1[:])
        is_sb = sb1.tile([D, 1], F32)
        nc.gpsimd.dma_start(out=is_sb[:], in_=is_in[:])
        ones_sb = sb1.tile([D, 1], F32)
        nc.vector.memset(ones_sb[:], 1.0)

        # layer-0 stationary (X): one big SWDGE load per peer
        stat0 = []
        for r in range(NCORES):
            t_ = stat0_pool.tile([128, NQ * NT, 128], E5, name=f"sx{r}", tag="s0")
            nc.gpsimd.dma_start(out=t_[:], in_=x_in[r * 128 : (r + 1) * 128, :])
            stat0.append(t_)

        # Exchange: 3 AllGathers. Chunks 0+1 share one op (the cc chain start
        # is gated by the runtime comm-init barrier, so chunk 1's data is
        # ready before the chain could start anyway — merging saves one
        # ~13us cc-queue overhead). Chunk 3's bounce carries sigma as a
        # 4-term e5m2 value split in 4 extra columns (~12-bit accurate).
        h_tb01 = dram.tile([128, 2 * IC], E5, name="h_tb01")
        h_tb2 = dram.tile([128, IC], E5, name="h_tb2")
        h_tb3 = dram.tile([128, IC + 4], E5, name="h_tb3")
        h_ag01 = dram.tile(
            [NCORES * 128, 2 * IC], E5, addr_space="Shared", name="h_ag01"
        )
        h_ag2 = dram.tile([NCORES * 128, IC], E5, addr_space="Shared", name="h_ag2")
        h_ag3 = dram.tile(
            [NCORES * 128, IC + 4], E5, addr_space="Shared", name="h_ag3"
        )
        h_bounce = [
            (h_tb01, 0, None),
            (h_tb01, IC, h_ag01),
            (h_tb2, 0, h_ag2),
            (h_tb3, 0, h_ag3),
        ]

        a_cache = {}

        def layer(stat_ap, w_sb, bias_mk, is_l1):
            bias = None
            hw_i = 0
            for c in range(NCH):
                agg = agg_pool.tile([128, IC], F32, name="ps", tag="ps")
                if not is_l1 and c == 0:
                    sig = sig_pool.tile([128, 1], F32, name="sg", tag="sg")
                for q in range(NQ):
                    blk = c * NQ + q
                    if is_l1 and c == NCH - 1:
                        at = a_cache[q]  # resident since layer 0
                    else:
                        if not is_l1 and c == NCH - 1:
                            at = ac_pool.tile(
                                [128, NCORES * NT, IC], E4, name=f"ac{q}", tag="ac"
                            )
                            a_cache[q] = at
                        else:
                            at = a_pool.tile(
                                [128, NCORES * NT, IC], E4, name="at", tag="at"
                            )
                        # layer-0 q3 blocks (except the cached c3) ride the
                        # third (SWDGE) lane; the PE needs q3 last per chunk
                        if not is_l1 and q == NQ - 1 and c < NCH - 1:
                            eng = nc.gpsimd
                        else:
                            eng = nc.sync if hw_i % 2 == 0 else nc.scalar
                            hw_i += 1
                        eng.dma_start(
                            out=at[:], in_=a_in[blk * 128 : (blk + 1) * 128, :]
                        )
                    for r in range(NCORES):
                        for tp in range(0, NT, 2):
                            first = q == 0 and r == 0 and tp == 0
                            last = q == NQ - 1 and r == NCORES - 1 and tp == NT - 2
                            nc.tensor.matmul(
                                agg[:],
                                lhsT=stat_ap(q, r, tp),
                                rhs=at[:, r * NT + tp : r * NT + tp + 2, :],
                                start=first,
                                stop=last,
                                perf_mode=DR,
                            )
                if bias is None:
                    bias = bias_mk()
                mt = m_pool.tile([128, IC], F16, name="mt", tag="mt")
                nc.vector.tensor_scalar(
                    out=mt[:], in0=agg[:], scalar1=bias[:], scalar2=is_sb[:],
                    op0=add, op1=mult,
                )
                if not is_l1:
                    hc = h_pool.tile([128, IC], E5, name="hc", tag="hc")
                else:
                    hcf = h_pool.tile([128, NT * 128], F32, name="hcf", tag="hcf")
                for t in range(NT):
                    lp = lin_pool.tile([128, D], F32, name="lp", tag="lp")
                    nc.tensor.matmul(
                        lp[:],
                        lhsT=mt[:, t * 128 : (t + 1) * 128],
                        rhs=w_sb[:],
                        start=True,
                        stop=True,
                    )
                    if is_l1:
                        nc.scalar.activation(
                            hcf[:, t * 128 : (t + 1) * 128], lp[:], relu
                        )
                    else:
                        ht = h_pool.tile([128, D], F32, name="ht", tag="ht")
                        nc.scalar.activation(ht[:], lp[:], relu)
                        # colsum(H) accumulation (fp32, width-1)
                        nc.tensor.matmul(
                            sig[:],
                            lhsT=ht[:],
                            rhs=ones_sb[:],
                            start=(c == 0 and t == 0),
                            stop=(c == NCH - 1 and t == NT - 1),
                        )
                        nc.vector.tensor_copy(
                            out=hc[:, t * 128 : (t + 1) * 128], in_=ht[:]
                        )
                if is_l1:
                    # one SWDGE write per chunk; host de-interleaves rows
                    nc.gpsimd.dma_start(
                        out=h_out[c * 128 : (c + 1) * 128, :], in_=hcf[:]
                    )
                if not is_l1:
                    import concourse.mybir as _mb

                    tb, off, ag = h_bounce[c]
                    nc.gpsimd.dma_start(out=tb[:, off : off + IC], in_=hc[:])
                    if c == NCH - 1:
                        # sigma 4-term e5m2 value split: sum(s4 cols) ~= sigma
                        res = sg_pool.tile([D, 1], F32, name="sp")
                        nc.vector.tensor_copy(out=res[:], in_=sig[:])
                        s4 = sg_pool.tile([D, 4], E5, name="s4")
                        for k in range(4):
                            nc.vector.tensor_copy(out=s4[:, k : k + 1], in_=res[:])
                            if k < 3:
                                vk = sg_pool.tile([D, 1], F32, name=f"vk{k}")
                                nc.vector.tensor_copy(
                                    out=vk[:], in_=s4[:, k : k + 1]
                                )
                                res2 = sg_pool.tile([D, 1], F32, name=f"rs{k}")
                                nc.vector.tensor_tensor(
                                    out=res2[:], in0=res[:], in1=vk[:],
                                    op=mybir.AluOpType.subtract,
                                )
                                res = res2
                        nc.gpsimd.dma_start(out=tb[:, IC : IC + 4], in_=s4[:])
                    if ag is not None:
                        nc.gpsimd.collective_compute(
                            "AllGather",
                            _mb.AluOpType.bypass,
                            replica_groups=[list(range(NCORES))],
                            ins=[tb[:]],
                            outs=[ag[:]],
                        )

        layer(
            lambda q, r, tp: stat0[r][:, q * NT + tp : q * NT + tp + 2, :],
            w0_sb,
            lambda: b0_sb,
            is_l1=False,
        )

        # layer-1 stationary from the chunked AllGathers (SWDGE; each load
        # only waits on its own AllGather)
        ag_src = [
            (h_ag01, 0),
            (h_ag01, IC),
            (h_ag2, 0),
            (h_ag3, 0),
        ]
        stat1 = []
        for q in range(NQ):
            src, off = ag_src[q]
            row = []
            for r in range(NCORES):
                t_ = stat1_pool.tile([128, NT, 128], E5, name=f"sh{q}_{r}", tag="s1")
                nc.gpsimd.dma_start(
                    out=t_[:], in_=src[r * 128 : (r + 1) * 128, off : off + IC]
                )
                row.append(t_)
            stat1.append(row)

        def mk_bias1():
            # accumulator and outputs live in the persistent pool (the sg
            # ring recycles buffers; acc must outlive the whole reduction)
            acc = sb1.tile([D, 1], F32, name="sacc")
            for r in range(NCORES):
                pt = sg_pool.tile([D, 4], E5, name=f"sg{r}")
                nc.gpsimd.dma_start(
                    out=pt[:], in_=h_ag3[r * 128 : (r + 1) * 128, IC : IC + 4]
                )
                pa = sg_pool.tile([D, 1], F32, name=f"pa{r}")
                nc.vector.tensor_tensor(
                    out=pa[:], in0=pt[:, 0:1], in1=pt[:, 1:2], op=add
                )
                pb = sg_pool.tile([D, 1], F32, name=f"pb{r}")
                nc.vector.tensor_tensor(
                    out=pb[:], in0=pt[:, 2:3], in1=pt[:, 3:4], op=add
                )
                if r == 0:
                    nc.vector.tensor_tensor(out=acc[:], in0=pa[:], in1=pb[:], op=add)
                else:
                    pr = sg_pool.tile([D, 1], F32, name=f"sa{r}")
                    nc.vector.tensor_tensor(out=pr[:], in0=pa[:], in1=pb[:], op=add)
                    nc.vector.tensor_tensor(
                        out=acc[:], in0=acc[:], in1=pr[:], op=add
                    )
            smu = sb1.tile([D, 1], F32, name="smu")
            nc.vector.tensor_tensor(
                out=smu[:], in0=c1_sb[:, 0:1], in1=c1_sb[:, 1:2], op=add
            )
            bias1 = sb1.tile([D, 1], F32, name="bias1")
            nc.vector.tensor_tensor(out=bias1[:], in0=acc[:], in1=smu[:], op=mult)
            return bias1

        layer(
            lambda q, r, tp: stat1[q][r][:, tp : tp + 2, :],
            w1_sb,
            mk_bias1,
            is_l1=True,
        )

    nc.finalize()
    return nc


def shard_inputs(A_norm, X, W0, W1, precision=None):
    """Host-side prep. Returns per-core input maps (complete, incl. weights)."""
    import ml_dtypes

    e4m3 = ml_dtypes.float8_e4m3
    e5m2 = ml_dtypes.float8_e5m2
    bf16 = ml_dtypes.bfloat16
    N = N_NODES
    mu = np.float32(1.0 / N)

    R = A_norm.astype(np.float32) - mu
    s = float(ml_dtypes.finfo(e4m3).max) / float(np.abs(R).max())
    Rq = (R * np.float32(s)).astype(e4m3)
    del R

    # peer-major layout: x0[r*128+p, (q*4+t)*128+d] = X[r*2048+q*512+t*128+p, d]
    x0 = np.ascontiguousarray(
        X.astype(e5m2).reshape(NCORES, NQ * NT, 128, D)
        .transpose(0, 2, 1, 3)
        .reshape(NCORES * 128, ROWS)
    )

    smu = np.float32(s * mu)
    hi = bf16(smu)
    lo = bf16(np.float32(smu - np.float32(hi)))
    c1 = np.broadcast_to(np.array([hi, lo], dtype=bf16), (D, 2)).copy()
    # TRUE colsum of X (float64) — kills the rank-1 projection of X's
    # quantization noise
    b0 = (
        np.float64(s) * np.float64(mu) * X.astype(np.float64).sum(axis=0)
    ).astype(np.float32).reshape(D, 1)
    is_ = np.full((D, 1), np.float32(1.0) / np.float32(s), dtype=np.float32)
    w0 = W0.astype(np.float16)
    w1 = W1.astype(np.float16)

    in_maps = []
    for core in range(NCORES):
        Rt = Rq[core * ROWS : (core + 1) * ROWS, :].T  # [16384 nodes, 2048]
        # [r, q, t, p, c, cc] -> rows (c*4+q)*128+p, cols (r*4+t)*512+cc
        a0 = np.ascontiguousarray(
            Rt.reshape(NCORES, NQ, NT, 128, NCH, IC)
            .transpose(4, 1, 3, 0, 2, 5)
            .reshape(NCH * NQ * 128, NCORES * NT * IC)
        )
        in_maps.append(
            {"a0": a0, "x0": x0, "w0": w0, "w1": w1, "b0": b0, "c1": c1, "is_": is_}
        )
    return in_maps


_CACHED = {}


def kernel(A_norm, X, W0, W1):
    A_norm = np.ascontiguousarray(A_norm, dtype=np.float32)
    X = np.ascontiguousarray(X, dtype=np.float32)
    W0 = np.ascontiguousarray(W0, dtype=np.float32)
    W1 = np.ascontiguousarray(W1, dtype=np.float32)

    from concourse.bass_utils import run_bass_kernel_spmd

    if PRECISION not in _CACHED:
        _CACHED[PRECISION] = build_gcn()
    nc = _CACHED[PRECISION]

    in_maps = shard_inputs(A_norm, X, W0, W1)
    res = run_bass_kernel_spmd(nc, in_maps, core_ids=list(range(NCORES)))
    outs = []
    for c in range(NCORES):
        o = res.results[c]["h_out"]  # [NCH*128, NT*128] chunk-major
        outs.append(
            o.reshape(NCH, 128, NT, D).transpose(0, 2, 1, 3).reshape(ROWS, D)
        )
    return np.concatenate(outs, axis=0)
